# revision 61
# baseline (speedup 1.0000x reference)
"""Trainium2 Bass kernel for nn_Cov_EBFLayer.

Math: out[b,o] = exp(-quad[o,b]),
  quad[o,b] = diff^T P_o diff,  diff = c_o - x_b,  P_o = B_o B_o^T  (PSD Gram)
            = sum_{d,f} P[o,d,f] x_d x_f - 2 v_o^T x + q3_o,  v = P c, q3 = c^T P c

Kernel strategy (per core, batch-sharded 8 x 1024):
  Symmetric-pair feature map over cyclic offsets: unordered pairs {d, f} at
  cyclic distance k are covered once by offset-k rows (d, (d+k)%64), k=1..32.
  17 feature chunks of 128 rows x 1024 batch, produced into dedicated SBUF
  tiles by three parallel lanes (engine balancing):
    S:  DVE tensor_mul of xb2=[x;x] against a host-prerotated slot operand
    P:  same, on GPSIMD
    A:  PE indicator matmul s = x_d + x_f -> PSUM; ACT squares it
        (u-features (x_d+x_f)^2, host folds x^2 terms into diag weights)
    M:  misc chunk rows 0:64 = x_d^2 (DVE), rows 64:128 = x_d (DVE copy)
  The PE main stream (~216 ns per N=512 matmul, 1 col/cycle warm) is the
  kernel bottleneck: 2 o-halves x 2 b-tiles x 17 chunks = 68 accumulating
  matmuls. The b-tiles run as two passes so the first half of the epilogue
  (ACT Exp + output DMA) overlaps the second pass.
Host does weight prep (P = beta beta^T, W layout, v, q3; O(model)) and
layout-only data movement (x transpose + rotated copies).
"""

import sys
from contextlib import ExitStack

import numpy as np

sys.path.insert(0, "/opt/trn_rl_repo")

import concourse.bass as bass  # noqa: E402
import concourse.tile as tile  # noqa: E402
from concourse import bacc, mybir  # noqa: E402
from concourse import bass_utils  # noqa: E402
from concourse._compat import with_exitstack  # noqa: E402

B, D, O, NCORES = 8192, 64, 256, 8
BSH = B // NCORES  # 1024 per-core batch shard
BT = 512  # matmul free-dim tile (one PSUM bank of fp32)
F32 = mybir.dt.float32
F16 = mybir.dt.float16

# Chunk sequence in ACCUMULATION (consumption) order. Pair chunk at list
# position j (skipping misc) gets cyclic offsets (2j+1, 2j+2).
_PATTERN = ["M", "A", "A", "S", "A", "S", "A", "S", "A", "P", "A", "S", "P", "S", "S", "S", "S"]
SEQ = []
_pj = 0
for _p in _PATTERN:
    if _p == "M":
        SEQ.append(("M", 0, 0))
    else:
        SEQ.append((_p, 2 * _pj + 1, 2 * _pj + 2))
        _pj += 1
NCH = len(SEQ)  # 17
assert sum(1 for s in SEQ if s[0] == "A") == 6
assert sum(1 for s in SEQ if s[0] == "S") == 8
assert sum(1 for s in SEQ if s[0] == "P") == 2
# b-tiles interleaved per chunk: 4 matmuls per ready chunk keeps the PE
# busier during the production-limited phase (HAM stays warm)
IND_ORDER = [j for j, s in enumerate(SEQ) if s[0] == "A"]
NACT = len(IND_ORDER)
# slot chunks in production order: pool first (slowest muls)
SLOT_ORDER = [j for j, s in enumerate(SEQ) if s[0] == "P"] + [
    j for j, s in enumerate(SEQ) if s[0] == "S"
]
NSLOT = len(SLOT_ORDER)
NSA = 5  # slots in the early DMA batch (2 pool + 3 early S chunks)
W1CH = 6  # chunks in the first W transfer (so main matmuls start early)
WCOLS = NCH * O + 4  # + 4 fp16 cols holding the fp32 -q3 bias (bitcast)


@with_exitstack
def _kernel(ctx: ExitStack, tc, outT, xT, wts, ind, xslots):
    nc = tc.nc

    cpool = ctx.enter_context(tc.tile_pool(name="const", bufs=1))
    opool = ctx.enter_context(tc.tile_pool(name="outs", bufs=4))
    qpool = ctx.enter_context(tc.tile_pool(name="psum_q", bufs=2, space="PSUM"))
    spool = ctx.enter_context(tc.tile_pool(name="psum_s", bufs=2, space="PSUM"))

    # ---- resident inputs; order sets DMA trigger + drain priority ----
    xb2 = cpool.tile([128, BSH], F16)  # [x; x] stacked
    nc.sync.dma_start(xb2[0:D, :], xT[:])
    i_sb = cpool.tile([D, NACT * 128], F16)
    nc.sync.dma_start(i_sb[:], ind[:])
    w_sb = cpool.tile([128, WCOLS], F16)
    slots_sb = cpool.tile([128, NSLOT * BSH], F16)
    # slot transfers trigger from GPSIMD (SWDGE): keeps them off the serial
    # SP trigger chain, so they start in parallel with the W/indicator DMAs
    nc.gpsimd.dma_start(slots_sb[:, 0 : NSA * BSH], xslots[:, 0 : NSA * BSH])
    nc.gpsimd.dma_start(slots_sb[:, NSA * BSH :], xslots[:, NSA * BSH :])
    nc.sync.dma_start(w_sb[:, 0 : W1CH * O], wts[:, 0 : W1CH * O])
    nc.sync.dma_start(w_sb[:, W1CH * O :], wts[:, W1CH * O :])
    b_sb = w_sb[:, NCH * O : NCH * O + 4].bitcast(F32)  # [128, 2] -q3

    # duplicate x rows on DVE (4x copy) instead of re-reading HBM
    g_misc = cpool.tile([128, BSH], F16)  # [x^2; x]
    nc.vector.tensor_copy(xb2[D : 2 * D, :], xb2[0:D, :])
    nc.vector.tensor_copy(g_misc[D:128, :], xb2[0:D, :])

    # quad accumulators: one 2-bank PSUM tile per o-half (bank per b-tile)
    pq = [
        qpool.tile([128, BSH], F32, name=f"pq{oh}", tag="pq") for oh in range(2)
    ]

    # dedicated G tile per chunk: production fully decoupled from consumption
    g_tiles = {}
    for j, (p, _, _) in enumerate(SEQ):
        g_tiles[j] = g_misc if p == "M" else cpool.tile([128, BSH], F16, name=f"g{j}")

    slot_of = {j: si for si, j in enumerate(SLOT_ORDER)}
    s_tiles = {}
    state = {"ind_ptr": 0, "done": 0}

    def top_up_inds():
        # keep <=2 chunks of indicator matmuls in flight
        while state["ind_ptr"] < NACT and state["ind_ptr"] - state["done"] < 2:
            ai = state["ind_ptr"]
            j = IND_ORDER[ai]
            s = spool.tile([128, BSH], F32, tag="s")
            for bt in range(2):
                nc.tensor.matmul(
                    s[:, bt * BT : (bt + 1) * BT],
                    i_sb[:, ai * 128 : (ai + 1) * 128],
                    xb2[0:D, bt * BT : (bt + 1) * BT],
                    start=True,
                    stop=True,
                )
            s_tiles[j] = s
            state["ind_ptr"] += 1

    # first indicator matmuls go ahead of the warm-up so ACT starts early
    top_up_inds()

    # ---- PE warm-up (HAM): keeps the PE busy until the first G is ready ----
    for i in range(4):
        nc.tensor.matmul(
            pq[i % 2][:, 0:256],
            xb2[0:D, 0:128],
            xb2[0:D, 0:256],
            start=True,
            stop=True,
        )

    # misc diag half: x_d^2
    nc.vector.tensor_mul(g_misc[0:D, :], xb2[0:D, :], xb2[0:D, :])

    def emit_production(j):
        p = SEQ[j][0]
        g = g_tiles[j]
        if p in ("S", "P"):
            eng = nc.vector if p == "S" else nc.gpsimd
            si = slot_of[j]
            eng.tensor_mul(g[:], xb2[:], slots_sb[:, si * BSH : (si + 1) * BSH])
        elif p == "A":  # wide square straight out of 2-bank PSUM
            nc.scalar.activation(
                g[:], s_tiles[j][:], mybir.ActivationFunctionType.Square
            )
            state["done"] += 1

    # front-load production of slot chunks (engines run as data arrives)
    for j in SLOT_ORDER:
        emit_production(j)

    def main_mm(j, bt):
        for oh in range(2):
            nc.tensor.matmul(
                pq[oh][:, bt * BT : (bt + 1) * BT],
                w_sb[:, j * O + oh * 128 : j * O + oh * 128 + 128],
                g_tiles[j][:, bt * BT : (bt + 1) * BT],
                start=(j == 0),
                stop=(j == NCH - 1),
            )

    def epilogue(bt):
        for oh in range(2):
            ob = opool.tile([128, BT], F16, tag="ob")
            nc.scalar.activation(
                ob[:],
                pq[oh][:, bt * BT : (bt + 1) * BT],
                mybir.ActivationFunctionType.Exp,
                bias=b_sb[:, oh : oh + 1],
                scale=-1.0,
            )
            nc.sync.dma_start(
                outT[oh * 128 : (oh + 1) * 128, bt * BT : (bt + 1) * BT], ob[:]
            )

    # ---- main loop: A-chunk production interleaved, 4 matmuls per chunk ----
    for j, (p, _, _) in enumerate(SEQ):
        top_up_inds()
        if p == "A":
            emit_production(j)
            top_up_inds()
        main_mm(j, 0)
        main_mm(j, 1)
    epilogue(0)
    epilogue(1)


_CACHE = {}


def _build():
    if "nc" in _CACHE:
        return _CACHE["nc"], _CACHE["aps"]
    nc = bacc.Bacc(
        "TRN2", target_bir_lowering=False, debug=False, num_devices=NCORES
    )
    xT = nc.dram_tensor("xT", [D, BSH], F16, kind="ExternalInput").ap()
    wts = nc.dram_tensor("wts", [128, WCOLS], F16, kind="ExternalInput").ap()
    ind = nc.dram_tensor("ind", [D, NACT * 128], F16, kind="ExternalInput").ap()
    xslots = nc.dram_tensor(
        "xslots", [128, NSLOT * BSH], F16, kind="ExternalInput"
    ).ap()
    outT = nc.dram_tensor("outT", [O, BSH], F16, kind="ExternalOutput").ap()
    with tile.TileContext(nc) as tc:
        _kernel(tc, outT, xT, wts, ind, xslots)
    nc.compile()
    _CACHE["nc"] = nc
    _CACHE["aps"] = (xT, wts, ind, xslots, outT)
    return nc, _CACHE["aps"]


def _host_prep(x, centers, betas):
    x32 = np.asarray(x, np.float32)
    betas32 = np.asarray(betas, np.float32)
    cen = np.asarray(centers, np.float32).reshape(O, D)
    # weight prep: O(model), batch-independent
    P = np.matmul(betas32, betas32.transpose(0, 2, 1))  # [O, D, D]
    w = np.einsum("ofe,of->oe", betas32, cen)
    v = np.einsum("ode,oe->od", betas32, w)
    q3 = np.einsum("oe,oe->o", w, w)

    dd = np.arange(D)
    R = np.zeros((O, D), np.float32)  # x^2 corrections from A-chunks
    Wstack = np.zeros((NCH, 128, O), np.float32)
    Imap = {}
    for j, (p, k1, k2) in enumerate(SEQ):
        if p == "M":
            continue
        for half, k in ((0, k1), (1, k2)):
            f = (dd + k) % D
            coeff = (2.0 if k < D // 2 else 1.0) * P[:, dd, f]  # [O, 64]
            if p == "A":
                A_ = coeff * 0.5
                Wstack[j, half * D : (half + 1) * D, :] = A_.T
                R[:, dd] += A_
                R[:, f] += A_  # f is a permutation: indices unique
            else:
                Wstack[j, half * D : (half + 1) * D, :] = coeff.T
        if p == "A":
            I = np.zeros((D, 128), np.float32)
            pp = np.arange(128)
            dcol = pp % D
            kcol = np.where(pp < D, k1, k2)
            I[dcol, pp] += 1.0
            I[(dcol + kcol) % D, pp] += 1.0
            Imap[j] = I
    mj = next(j for j, s in enumerate(SEQ) if s[0] == "M")
    Wstack[mj, 0:D, :] = (P[:, dd, dd] - R).T
    Wstack[mj, D:128, :] = (-2.0 * v).T

    wts = np.empty((128, WCOLS), np.float16)
    wts[:, 0 : NCH * O] = (
        Wstack.transpose(1, 0, 2).reshape(128, NCH * O).astype(np.float16)
    )
    # -q3 bias: fp32 bits carried in 4 fp16 columns
    biasf = np.ascontiguousarray((-q3).reshape(2, 128).T.astype(np.float32))
    wts[:, NCH * O : NCH * O + 4] = biasf.view(np.float16)
    wts = np.ascontiguousarray(wts)
    ind = np.ascontiguousarray(
        np.concatenate([Imap[j] for j in IND_ORDER], axis=1)
    ).astype(np.float16)

    xT_shards = []
    xslot_shards = []
    for i in range(NCORES):
        xTi = np.ascontiguousarray(
            x32[i * BSH : (i + 1) * BSH].T
        ).astype(np.float16)
        xT_shards.append(xTi)
        # rotated slot operands, in production order (layout-only gather)
        sl = np.empty((128, NSLOT, BSH), np.float16)
        for si, j in enumerate(SLOT_ORDER):
            _, k1, k2 = SEQ[j]
            sl[0:D, si, :] = np.roll(xTi, -k1, axis=0)
            sl[D:128, si, :] = np.roll(xTi, -k2, axis=0)
        xslot_shards.append(np.ascontiguousarray(sl.reshape(128, NSLOT * BSH)))
    return xT_shards, xslot_shards, wts, ind


def _run(x, centers, betas, trace=False):
    nc, (xT, wts_ap, ind_ap, xslots_ap, outT) = _build()
    xT_shards, xslot_shards, wts, ind = _host_prep(x, centers, betas)
    in_maps = [
        {
            xT.name: xT_shards[i],
            wts_ap.name: wts,
            ind_ap.name: ind,
            xslots_ap.name: xslot_shards[i],
        }
        for i in range(NCORES)
    ]
    res = bass_utils.run_bass_kernel_spmd(
        nc, in_maps, core_ids=list(range(NCORES)), trace=trace
    )
    out = np.concatenate(
        [np.asarray(res.results[i][outT.name]).T for i in range(NCORES)],
        axis=0,
    )
    return out.astype(np.float32), res


def kernel(x, centers, betas):
    out, _ = _run(x, centers, betas, trace=False)
    return out


# revision 62
# speedup vs baseline: 1.3039x; 1.3039x over previous
"""Trainium2 Bass kernel for nn_Cov_EBFLayer.

Math: out[b,o] = exp(-quad[o,b]),
  quad[o,b] = diff^T P_o diff,  diff = c_o - x_b,  P_o = B_o B_o^T  (PSD Gram)
            = sum_{d,f} P[o,d,f] x_d x_f - 2 v_o^T x + q3_o,  v = P c, q3 = c^T P c

Kernel strategy (per core, batch-sharded 8 x 1024):
  Symmetric-pair feature map over cyclic offsets: unordered pairs {d, f} at
  cyclic distance k are covered once by offset-k rows (d, (d+k)%64), k=1..32.
  17 feature chunks of 128 rows x 1024 batch, produced into dedicated SBUF
  tiles by three parallel lanes (engine balancing):
    S:  DVE tensor_mul of xb2=[x;x] against a host-prerotated slot operand
    P:  same, on GPSIMD
    A:  PE indicator matmul s = x_d + x_f -> PSUM; ACT squares it
        (u-features (x_d+x_f)^2, host folds x^2 terms into diag weights)
    M:  misc chunk rows 0:64 = x_d^2 (DVE), rows 64:128 = x_d (DVE copy)
  The PE main stream (~216 ns per N=512 matmul, 1 col/cycle warm) is the
  kernel bottleneck: 2 o-halves x 2 b-tiles x 17 chunks = 68 accumulating
  matmuls. The b-tiles run as two passes so the first half of the epilogue
  (ACT Exp + output DMA) overlaps the second pass.
Host does weight prep (P = beta beta^T, W layout, v, q3; O(model)) and
layout-only data movement (x transpose + rotated copies).
"""

import sys
from contextlib import ExitStack

import numpy as np

sys.path.insert(0, "/opt/trn_rl_repo")

import concourse.bass as bass  # noqa: E402
import concourse.tile as tile  # noqa: E402
from concourse import bacc, mybir  # noqa: E402
from concourse import bass_utils  # noqa: E402
from concourse._compat import with_exitstack  # noqa: E402

B, D, O, NCORES = 8192, 64, 256, 8
BSH = B // NCORES  # 1024 per-core batch shard
BT = 512  # matmul free-dim tile (one PSUM bank of fp32)
F32 = mybir.dt.float32
F16 = mybir.dt.float16

# Chunk sequence in ACCUMULATION (consumption) order. Pair chunk at list
# position j (skipping misc) gets cyclic offsets (2j+1, 2j+2).
_PATTERN = ["M", "A", "A", "S", "A", "S", "A", "S", "A", "P", "A", "S", "P", "S", "S", "S", "S"]
SEQ = []
_pj = 0
for _p in _PATTERN:
    if _p == "M":
        SEQ.append(("M", 0, 0))
    else:
        SEQ.append((_p, 2 * _pj + 1, 2 * _pj + 2))
        _pj += 1
NCH = len(SEQ)  # 17
assert sum(1 for s in SEQ if s[0] == "A") == 6
assert sum(1 for s in SEQ if s[0] == "S") == 8
assert sum(1 for s in SEQ if s[0] == "P") == 2
# b-tiles interleaved per chunk: 4 matmuls per ready chunk keeps the PE
# busier during the production-limited phase (HAM stays warm)
IND_ORDER = [j for j, s in enumerate(SEQ) if s[0] == "A"]
NACT = len(IND_ORDER)
# slot chunks in production order: pool first (slowest muls)
SLOT_ORDER = [j for j, s in enumerate(SEQ) if s[0] == "P"] + [
    j for j, s in enumerate(SEQ) if s[0] == "S"
]
NSLOT = len(SLOT_ORDER)
NSA = 5  # slots in the early DMA batch (2 pool + 3 early S chunks)
W1CH = 6  # chunks in the first W transfer (so main matmuls start early)
WCOLS = NCH * O + 4  # + 4 fp16 cols holding the fp32 -q3 bias (bitcast)


@with_exitstack
def _kernel(ctx: ExitStack, tc, outT, xT, wts, ind, xslots):
    nc = tc.nc

    cpool = ctx.enter_context(tc.tile_pool(name="const", bufs=1))
    opool = ctx.enter_context(tc.tile_pool(name="outs", bufs=4))
    qpool = ctx.enter_context(tc.tile_pool(name="psum_q", bufs=2, space="PSUM"))
    spool = ctx.enter_context(tc.tile_pool(name="psum_s", bufs=2, space="PSUM"))

    # ---- resident inputs; order sets DMA trigger + drain priority ----
    xb2 = cpool.tile([128, BSH], F16)  # [x; x] stacked
    nc.sync.dma_start(xb2[0:D, :], xT[:])
    i_sb = cpool.tile([D, NACT * 128], F16)
    nc.sync.dma_start(i_sb[:], ind[:])
    w_sb = cpool.tile([128, WCOLS], F16)
    slots_sb = cpool.tile([128, NSLOT * BSH], F16)
    nc.sync.dma_start(w_sb[:, 0 : W1CH * O], wts[:, 0 : W1CH * O])
    nc.sync.dma_start(slots_sb[:, 0 : NSA * BSH], xslots[:, 0 : NSA * BSH])
    nc.sync.dma_start(w_sb[:, W1CH * O :], wts[:, W1CH * O :])
    nc.sync.dma_start(slots_sb[:, NSA * BSH :], xslots[:, NSA * BSH :])
    b_sb = w_sb[:, NCH * O : NCH * O + 4].bitcast(F32)  # [128, 2] -q3

    # duplicate x rows on DVE (4x copy) instead of re-reading HBM
    g_misc = cpool.tile([128, BSH], F16)  # [x^2; x]
    nc.vector.tensor_copy(xb2[D : 2 * D, :], xb2[0:D, :])
    nc.vector.tensor_copy(g_misc[D:128, :], xb2[0:D, :])

    # quad accumulators: one 2-bank PSUM tile per o-half (bank per b-tile)
    pq = [
        qpool.tile([128, BSH], F32, name=f"pq{oh}", tag="pq") for oh in range(2)
    ]

    # dedicated G tile per chunk: production fully decoupled from consumption
    g_tiles = {}
    for j, (p, _, _) in enumerate(SEQ):
        g_tiles[j] = g_misc if p == "M" else cpool.tile([128, BSH], F16, name=f"g{j}")

    slot_of = {j: si for si, j in enumerate(SLOT_ORDER)}
    s_tiles = {}
    state = {"ind_ptr": 0, "done": 0}

    def top_up_inds():
        # keep <=2 chunks of indicator matmuls in flight
        while state["ind_ptr"] < NACT and state["ind_ptr"] - state["done"] < 2:
            ai = state["ind_ptr"]
            j = IND_ORDER[ai]
            s = spool.tile([128, BSH], F32, tag="s")
            for bt in range(2):
                nc.tensor.matmul(
                    s[:, bt * BT : (bt + 1) * BT],
                    i_sb[:, ai * 128 : (ai + 1) * 128],
                    xb2[0:D, bt * BT : (bt + 1) * BT],
                    start=True,
                    stop=True,
                )
            s_tiles[j] = s
            state["ind_ptr"] += 1

    # first indicator matmuls go ahead of the warm-up so ACT starts early
    top_up_inds()

    # ---- PE warm-up (HAM): keeps the PE busy until the first G is ready ----
    for i in range(4):
        nc.tensor.matmul(
            pq[i % 2][:, 0:256],
            xb2[0:D, 0:128],
            xb2[0:D, 0:256],
            start=True,
            stop=True,
        )

    # misc diag half: x_d^2
    nc.vector.tensor_mul(g_misc[0:D, :], xb2[0:D, :], xb2[0:D, :])

    def emit_production(j):
        p = SEQ[j][0]
        g = g_tiles[j]
        if p in ("S", "P"):
            eng = nc.vector if p == "S" else nc.gpsimd
            si = slot_of[j]
            eng.tensor_mul(g[:], xb2[:], slots_sb[:, si * BSH : (si + 1) * BSH])
        elif p == "A":  # wide square straight out of 2-bank PSUM
            nc.scalar.activation(
                g[:], s_tiles[j][:], mybir.ActivationFunctionType.Square
            )
            state["done"] += 1

    # front-load production of slot chunks (engines run as data arrives)
    for j in SLOT_ORDER:
        emit_production(j)

    def main_mm(j, bt):
        for oh in range(2):
            nc.tensor.matmul(
                pq[oh][:, bt * BT : (bt + 1) * BT],
                w_sb[:, j * O + oh * 128 : j * O + oh * 128 + 128],
                g_tiles[j][:, bt * BT : (bt + 1) * BT],
                start=(j == 0),
                stop=(j == NCH - 1),
            )

    def epilogue(bt):
        for oh in range(2):
            ob = opool.tile([128, BT], F16, tag="ob")
            nc.scalar.activation(
                ob[:],
                pq[oh][:, bt * BT : (bt + 1) * BT],
                mybir.ActivationFunctionType.Exp,
                bias=b_sb[:, oh : oh + 1],
                scale=-1.0,
            )
            nc.sync.dma_start(
                outT[oh * 128 : (oh + 1) * 128, bt * BT : (bt + 1) * BT], ob[:]
            )

    # ---- main loop: A-chunk production interleaved, 4 matmuls per chunk ----
    for j, (p, _, _) in enumerate(SEQ):
        top_up_inds()
        if p == "A":
            emit_production(j)
            top_up_inds()
        main_mm(j, 0)
        main_mm(j, 1)
    epilogue(0)
    epilogue(1)


_CACHE = {}


def _build():
    if "nc" in _CACHE:
        return _CACHE["nc"], _CACHE["aps"]
    nc = bacc.Bacc(
        "TRN2", target_bir_lowering=False, debug=False, num_devices=NCORES
    )
    xT = nc.dram_tensor("xT", [D, BSH], F16, kind="ExternalInput").ap()
    wts = nc.dram_tensor("wts", [128, WCOLS], F16, kind="ExternalInput").ap()
    ind = nc.dram_tensor("ind", [D, NACT * 128], F16, kind="ExternalInput").ap()
    xslots = nc.dram_tensor(
        "xslots", [128, NSLOT * BSH], F16, kind="ExternalInput"
    ).ap()
    outT = nc.dram_tensor("outT", [O, BSH], F16, kind="ExternalOutput").ap()
    with tile.TileContext(nc) as tc:
        _kernel(tc, outT, xT, wts, ind, xslots)
    nc.compile()
    _CACHE["nc"] = nc
    _CACHE["aps"] = (xT, wts, ind, xslots, outT)
    return nc, _CACHE["aps"]


def _host_prep(x, centers, betas):
    x32 = np.asarray(x, np.float32)
    betas32 = np.asarray(betas, np.float32)
    cen = np.asarray(centers, np.float32).reshape(O, D)
    # weight prep: O(model), batch-independent
    P = np.matmul(betas32, betas32.transpose(0, 2, 1))  # [O, D, D]
    w = np.einsum("ofe,of->oe", betas32, cen)
    v = np.einsum("ode,oe->od", betas32, w)
    q3 = np.einsum("oe,oe->o", w, w)

    dd = np.arange(D)
    R = np.zeros((O, D), np.float32)  # x^2 corrections from A-chunks
    Wstack = np.zeros((NCH, 128, O), np.float32)
    Imap = {}
    for j, (p, k1, k2) in enumerate(SEQ):
        if p == "M":
            continue
        for half, k in ((0, k1), (1, k2)):
            f = (dd + k) % D
            coeff = (2.0 if k < D // 2 else 1.0) * P[:, dd, f]  # [O, 64]
            if p == "A":
                A_ = coeff * 0.5
                Wstack[j, half * D : (half + 1) * D, :] = A_.T
                R[:, dd] += A_
                R[:, f] += A_  # f is a permutation: indices unique
            else:
                Wstack[j, half * D : (half + 1) * D, :] = coeff.T
        if p == "A":
            I = np.zeros((D, 128), np.float32)
            pp = np.arange(128)
            dcol = pp % D
            kcol = np.where(pp < D, k1, k2)
            I[dcol, pp] += 1.0
            I[(dcol + kcol) % D, pp] += 1.0
            Imap[j] = I
    mj = next(j for j, s in enumerate(SEQ) if s[0] == "M")
    Wstack[mj, 0:D, :] = (P[:, dd, dd] - R).T
    Wstack[mj, D:128, :] = (-2.0 * v).T

    wts = np.empty((128, WCOLS), np.float16)
    wts[:, 0 : NCH * O] = (
        Wstack.transpose(1, 0, 2).reshape(128, NCH * O).astype(np.float16)
    )
    # -q3 bias: fp32 bits carried in 4 fp16 columns
    biasf = np.ascontiguousarray((-q3).reshape(2, 128).T.astype(np.float32))
    wts[:, NCH * O : NCH * O + 4] = biasf.view(np.float16)
    wts = np.ascontiguousarray(wts)
    ind = np.ascontiguousarray(
        np.concatenate([Imap[j] for j in IND_ORDER], axis=1)
    ).astype(np.float16)

    xT_shards = []
    xslot_shards = []
    for i in range(NCORES):
        xTi = np.ascontiguousarray(
            x32[i * BSH : (i + 1) * BSH].T
        ).astype(np.float16)
        xT_shards.append(xTi)
        # rotated slot operands, in production order (layout-only gather)
        sl = np.empty((128, NSLOT, BSH), np.float16)
        for si, j in enumerate(SLOT_ORDER):
            _, k1, k2 = SEQ[j]
            sl[0:D, si, :] = np.roll(xTi, -k1, axis=0)
            sl[D:128, si, :] = np.roll(xTi, -k2, axis=0)
        xslot_shards.append(np.ascontiguousarray(sl.reshape(128, NSLOT * BSH)))
    return xT_shards, xslot_shards, wts, ind


def _run(x, centers, betas, trace=False):
    nc, (xT, wts_ap, ind_ap, xslots_ap, outT) = _build()
    xT_shards, xslot_shards, wts, ind = _host_prep(x, centers, betas)
    in_maps = [
        {
            xT.name: xT_shards[i],
            wts_ap.name: wts,
            ind_ap.name: ind,
            xslots_ap.name: xslot_shards[i],
        }
        for i in range(NCORES)
    ]
    res = bass_utils.run_bass_kernel_spmd(
        nc, in_maps, core_ids=list(range(NCORES)), trace=trace
    )
    out = np.concatenate(
        [np.asarray(res.results[i][outT.name]).T for i in range(NCORES)],
        axis=0,
    )
    return out.astype(np.float32), res


def kernel(x, centers, betas):
    out, _ = _run(x, centers, betas, trace=False)
    return out
